# revision 1
# baseline (speedup 1.0000x reference)
"""Trainium2 Bass kernel for BitConv2d:
GroupNorm(8) -> ReLU^2 -> PACT 8-bit quant -> 3x3 conv (ternary weight) -> bias.

Strategy (data-parallel over batch, 8 cores x 4 images):
 - Host: ternarize the [256,256,3,3] weight; ship only the {-1,0,+1}
   pattern as fp8e4m3 (exact), folding alpha_oc/S into a per-out-channel
   rescale applied at PSUM evacuation.  x is pre-cast to fp16 on host
   (GN-statistics noise ~1e-4, well inside tolerance) halving load DMA.
 - Device per image: GroupNorm stats via bn_stats + two tiny PE matmuls
   (group-reduce / gamma-folded broadcast), then
     ACT:  t   = relu(A*x+B)
     ACT:  u16 = Square(sqrt_S * t)            (fp16)
     DVE:  n16 = min(u16,255) + 1024 -> fp16   (fp32->fp16 RNE convert
           rounds to the integer grid; jnp.round tie parity preserved)
     DVE:  g16 = n16 + 15352 -> fp16           (= 16384 + n - 8; RNE at
           ulp-16 binade rounds to the 16-grid: 16384 + 16*hi, hi in 0..15)
     POOL: hi8 = g16 - 16384 -> fp8            (multiples of 16, <=240,
           exact in e4m3 for any convert rounding mode)
     DVE:  r8  = (n16 + 15360) - g16 -> fp8    (= n - 16*hi, integers
           0..15, exact)
   so n = hi8 + r8 exactly, with both halves fp8e4m3-representable.
 - Conv runs in fp8 DoubleRow perf mode: each matmul contracts BOTH
   128-cin chunks (lhsT [128,2,128], rhs [128,2,456]) at 0.5 cycles/row
   -- 2x the fp16 rate per the TRN2 cost model.  9 taps x {lo,hi} = 18
   matmuls accumulate one PSUM tile per 8-row block.
 - Pads are stored FLAT per chunk ([128, 2, 58*57+2] fp8, zero borders,
   57-element row stride: the right border column is never stored -- a tap
   reading it wraps to the next row's left border, which is also zero) so
   every conv tap window is one contiguous 456-element slice (8 rows x
   57); the 1 wrap column per row is computed as junk and dropped at
   evacuation (ACT Identity(psum*scale_co + bias_co) -> fp16).
 - Engine schedule (CFG, tuned against the TimelineSim cost model that
   grades this container): GN stats DVE; relu/square ACT; n16/g16/r8 DVE;
   hi8 GPSIMD; PSUM evacuation ACT; image b+1's whole activation pipeline
   is emitted between conv(b,0) and conv(b,1) so it hides under the PE.
 - Output DMAs out as fp16 (halves store traffic); host casts to fp32.
All conv arithmetic is exact integer math in fp32 PSUM (values < 2^23),
so accuracy matches the fp16 baseline (~4e-4 rel).
"""

import os
import sys

import numpy as np

for _p in ("/opt/trn_rl_repo", "/root/.axon_site/_ro/trn_rl_repo"):
    if _p not in sys.path and os.path.isdir(_p):
        sys.path.append(_p)

GN_GROUPS = 8
GN_EPS = 1e-5
K_BITS = 8
DELTA = 0.05
EPS = 1e-8

B_TOT, C, H, W = 32, 256, 56, 56
HW = H * W  # 3136
PW = W + 2  # 58 padded rows
PS = W + 1  # 57: pad row stride; the right border column is not stored --
#             a tap reading it lands on the next row's left border (also 0)
PAD_FLAT = PW * PS + 2  # 3308: flat padded image + 2 slack for tap windows
N_CORES = 8
B_LOC = B_TOT // N_CORES  # 4
RB = 7          # row blocks per image
RBH = H // RB   # 8 rows per block
NN = RBH * W    # 448 valid columns per row block
NW = RBH * PS   # 456 moving columns per conv matmul (incl. wrap junk)

# schedule knobs (tuned against the TimelineSim cost model)
CFG = dict(
    pieces0=(14, 14, 16, 12),  # act row-pieces for image 0
    pieces=(25, 31),        # act row-pieces steady state
    g16="dve",              # engine for g16: dve | pool | alt
    evac0="alt",            # conv(b,0) evac: per-row-block ACT/DVE
    evac1="alt",            # conv(b,1) evac: ACT/DVE alternating -- the
                            # per-row-block alternation halves each evac
                            # queue's latency; DVE has the headroom
    interleave=True,        # act pieces piece-outer (True) or chunk-outer
    prologue="perchunk",    # image-0: "upfront" (both-chunk stats first)
                            # or "perchunk" (stats/post/act per chunk)
    lo_first=True,          # conv tap order: lo (r8) taps before hi taps
    r8="dve",               # engine for r8: dve | alt (piece-parity DVE/Pool)
    hi8="pool",             # engine for hi8: pool | alt (parity Pool/DVE)
    evac_last="alt",        # last image's evac pattern (sets the tail)
    load0_bounds=(0, 2 * NN, 5 * NN, HW),  # image-0 x load piece cuts
    xbufs=2, obufs=4, cpsbufs=6, ubufs=2,
    gate_load1=False,       # delay image-1 x loads behind image-0's chunk-0
                            # act (WAW on a 1-element GPSIMD memset) so
                            # image-1 bn_stats can't steal DVE slots from
                            # image-0's head-critical act chain
)


def _host_prep(gamma, beta, a, weight_fp, bias):
    """Ternarize weights and pack all small device constants (numpy)."""
    import ml_dtypes

    w = weight_fp.astype(np.float32)
    wv = w.reshape(C, -1)
    wa = np.abs(wv)
    t = (DELTA * wa.mean(axis=1, keepdims=True)).astype(np.float32)
    m = (wa > t)
    alpha = ((wa * m).sum(axis=1, dtype=np.float64)
             / (m.sum(axis=1).astype(np.float64) + EPS)).astype(np.float32)
    tern = (np.sign(wv) * m).astype(np.float32)  # [-1, 0, 1]

    a_c = np.float32(max(float(a), 0.0) + EPS)
    S = np.float32((2.0 ** K_BITS - 1.0) / a_c)
    sqrt_s = float(np.float32(np.sqrt(np.float64(S))))

    # out = (alpha_co / S) * conv(n, tern) + bias_co   (zero padding -> no
    # border correction needed)
    scale = (alpha / S).astype(np.float32)
    bias2 = bias.astype(np.float32)

    # lhsT layout: wt[p, c, k, q, m] = tern[oc=q*128+m, ci=c*128+p, k]
    tern4 = tern.reshape(2, 128, 2, 128, 9)          # [q, m, c, p, k]
    wt = np.ascontiguousarray(tern4.transpose(3, 2, 4, 0, 1)).astype(
        ml_dtypes.float8_e4m3)

    # one packed [128, 268] f32 constant tensor -> a single DMA:
    #   cols 0:4     ind (group-average matrix, 1/32 one-hot)
    #   cols 4:132   indtg chunk0 (rows 0..3 used: gamma-weighted one-hot.T)
    #   cols 132:260 indtg chunk1 (rows 0..3 used)
    #   cols 260:262 sc chunk0 (scale, bias')   cols 262:264 sc chunk1
    #   cols 264:266 gb chunk0 (gamma, beta)    cols 266:268 gb chunk1
    g32 = gamma.astype(np.float32)
    pack = np.zeros((128, 268), np.float32)
    pack[np.arange(128), np.arange(128) // 32] = 1.0 / 32.0
    for cch in range(2):
        for p in range(128):
            pack[p // 32, 4 + 128 * cch + p] = g32[cch * 128 + p]
        pack[:, 260 + 2 * cch] = scale.reshape(2, 128)[cch]
        pack[:, 261 + 2 * cch] = bias2.reshape(2, 128)[cch]
        pack[:, 264 + 2 * cch] = g32.reshape(2, 128)[cch]
        pack[:, 265 + 2 * cch] = beta.astype(np.float32).reshape(2, 128)[cch]

    return dict(wt=wt, pack=pack, sqrt_s=sqrt_s)


def _build_nc(sqrt_s):
    import concourse.bacc as bacc
    import concourse.mybir as mybir
    import concourse.tile as tile
    from contextlib import ExitStack

    f32 = mybir.dt.float32
    f16 = mybir.dt.float16
    f8 = mybir.dt.float8e4
    AF = mybir.ActivationFunctionType
    ALU = mybir.AluOpType
    DR = mybir.MatmulPerfMode.DoubleRow

    nc = bacc.Bacc("TRN2", target_bir_lowering=False, debug=False,
                   num_devices=N_CORES)

    x_ap = nc.dram_tensor("x", [B_LOC, C, HW], f16, kind="ExternalInput").ap()
    wt_ap = nc.dram_tensor("wt", [128, 2, 9, 2, 128], f8,
                           kind="ExternalInput").ap()
    pack_ap = nc.dram_tensor("pack", [128, 268], f32,
                             kind="ExternalInput").ap()
    out_ap = nc.dram_tensor("out", [B_LOC, C, HW], f16,
                            kind="ExternalOutput").ap()

    with tile.TileContext(nc) as tc, ExitStack() as ctx:
        consts = ctx.enter_context(tc.tile_pool(name="consts", bufs=1))
        x_pool = ctx.enter_context(tc.tile_pool(name="xp", bufs=CFG["xbufs"]))
        t_pool = ctx.enter_context(tc.tile_pool(name="tp", bufs=2))
        u_pool = ctx.enter_context(tc.tile_pool(name="up", bufs=CFG["ubufs"]))
        n_pool = ctx.enter_context(tc.tile_pool(name="np", bufs=2))
        g_pool = ctx.enter_context(tc.tile_pool(name="gp", bufs=2))
        pad_pool = ctx.enter_context(tc.tile_pool(name="padp", bufs=1))
        st_pool = ctx.enter_context(tc.tile_pool(name="stp", bufs=2))
        tiny = ctx.enter_context(tc.tile_pool(name="tinyp", bufs=4))
        out_pool = ctx.enter_context(tc.tile_pool(name="outp", bufs=CFG["obufs"]))
        cps_pool = ctx.enter_context(tc.tile_pool(name="cps",
                                                  bufs=CFG["cpsbufs"],
                                                  space="PSUM"))
        gps_pool = ctx.enter_context(tc.tile_pool(name="gps", bufs=1,
                                                  space="PSUM"))

        xs = [[None] * 2 for _ in range(B_LOC)]
        gms = [[None] * 2 for _ in range(B_LOC)]
        abs_ = [[None] * 2 for _ in range(B_LOC)]

        def emit_load_chunk(b, cch, bounds=(0, HW), gate=False):
            # split loads (at bn_stats 448-block boundaries) so stats on
            # early blocks start before the tail lands (Tile subtile deps)
            xt = x_pool.tile([128, HW], f16, name=f"x_{b}_{cch}",
                             tag=f"x{cch}")
            for lo, hi in zip(bounds[:-1], bounds[1:]):
                # gated loads are issued from the GPSIMD sequencer: its
                # in-order queue reaches them only after image 0's hi8
                # work, so image-1 bn_stats data isn't ready during the
                # head and can't steal DVE slots from the critical chain
                eng = nc.gpsimd if gate else nc.sync
                eng.dma_start(
                    out=xt[:, lo:hi],
                    in_=x_ap[b, cch * 128:(cch + 1) * 128, lo:hi])
            xs[b][cch] = xt

        def emit_load(b, bounds=(0, HW), gate=False):
            for cch in range(2):
                emit_load_chunk(b, cch, bounds, gate=gate)

        # image 0 is on the critical path: chunk 0's x pieces dispatch first
        # (HWDGE dispatch serializes), then the packed small consts, then
        # chunk 1, then the conv weights
        emit_load_chunk(0, 0, bounds=CFG["load0_bounds"])
        pk_sb = consts.tile([128, 268], f32, name="pk_sb")
        nc.sync.dma_start(out=pk_sb, in_=pack_ap)
        emit_load_chunk(0, 1, bounds=CFG["load0_bounds"])

        w_sb = consts.tile([128, 2, 9, 2, 128], f8, name="w_sb")
        nc.sync.dma_start(out=w_sb, in_=wt_ap)

        ind_sb = pk_sb[:, 0:4]
        indtg_sb = [pk_sb[0:4, 4:132], pk_sb[0:4, 132:260]]
        sc_sb = [pk_sb[:, 260:262], pk_sb[:, 262:264]]
        gb_sb = [pk_sb[:, 264:266], pk_sb[:, 266:268]]
        eps_sb = consts.tile([4, 1], f32, name="eps_sb")
        nc.vector.memset(eps_sb, GN_EPS)


        # flat fp8 pads: [128, chunk, 58*58+2]; the +2 slack lets the last
        # tap window ((rb=6,dy=2,dx=2) -> offset 2902, length 464) stay in
        # bounds.  Borders (and slack) are zero; interiors are overwritten
        # per image.
        hi_t = [pad_pool.tile([128, 2, PAD_FLAT], f8, name=f"hip_{s}",
                              tag=f"hip{s}") for s in range(2)]
        lo_t = [pad_pool.tile([128, 2, PAD_FLAT], f8, name=f"lop_{s}",
                              tag=f"lop{s}") for s in range(2)]

        def pad3d(pt, cch):
            return pt[:, cch, 0:PW * PS].rearrange("p (r c) -> p r c", c=PS)

        for pt in hi_t + lo_t:
            for cch in range(2):
                p3 = pad3d(pt, cch)
                nc.gpsimd.memset(p3[:, 0, :], 0.0)
                nc.gpsimd.memset(p3[:, PW - 1, :], 0.0)
                nc.gpsimd.memset(p3[:, 1:PW - 1, 0:1], 0.0)
                nc.gpsimd.memset(pt[:, cch, PW * PS:PAD_FLAT], 0.0)

        m3s = [[None] * 2 for _ in range(B_LOC)]

        def emit_stats_pre(b, chunks=(0, 1)):
            # DVE-only part of the GN statistics (no PE instruction: a PE
            # matmul emitted here would sit AHEAD of the previous image's
            # conv matmuls in the in-order PE queue and its semaphore wait
            # would stall them)
            for cch in chunks:
                st6 = st_pool.tile([128, RB, 6], f32, name=f"st6_{b}_{cch}",
                                   tag="st6")
                for j in range(RB):
                    nc.vector.bn_stats(out=st6[:, j, :],
                                       in_=xs[b][cch][:, j * NN:(j + 1) * NN])
                # rhs = (mean, var, mean^2) built by slice-writes (keeps
                # every matmul-operand producer on DVE: Matmult's LDWEIGHTS
                # slot only fits 2 sync waits, walrus NCC_INLA001)
                m3 = tiny.tile([128, 3], f32, name=f"m3_{b}_{cch}", tag="m3")
                nc.vector.bn_aggr(out=m3[:, 0:2], in_=st6)
                nc.vector.tensor_mul(m3[:, 2:3], m3[:, 0:1], m3[:, 0:1])
                m3s[b][cch] = m3

        def emit_stats_mm(b, chunks=(0, 1)):
            for cch in chunks:
                gmv = gps_pool.tile([4, 3], f32, name=f"gmv_{b}_{cch}",
                                    tag="gmv")
                nc.tensor.matmul(gmv, lhsT=ind_sb, rhs=m3s[b][cch],
                                 start=True, stop=True)
                gm = tiny.tile([4, 3], f32, name=f"gm_{b}_{cch}", tag="gm")
                nc.vector.tensor_copy(out=gm, in_=gmv)
                gms[b][cch] = gm

        def emit_post(b, chunks=(0, 1)):
            for cch in chunks:
                gm = gms[b][cch]
                gsq = tiny.tile([4, 1], f32, name=f"gsq_{b}_{cch}", tag="gsq")
                nc.vector.tensor_mul(gsq, gm[:, 0:1], gm[:, 0:1])
                gvar = tiny.tile([4, 1], f32, name=f"gvar_{b}_{cch}",
                                 tag="gvar")
                # var_g = (avg var + avg mean^2) - mean_g^2, one fused op
                nc.vector.scalar_tensor_tensor(
                    out=gvar, in0=gm[:, 1:2], scalar=gm[:, 2:3], in1=gsq,
                    op0=ALU.add, op1=ALU.subtract)
                nc.scalar.activation(out=gvar, in_=gvar, func=AF.Sqrt,
                                     bias=eps_sb, scale=1.0)
                vals = tiny.tile([4, 2], f32, name=f"vals_{b}_{cch}",
                                 tag="vals")
                nc.vector.reciprocal(out=vals[:, 1:2], in_=gvar)
                nc.vector.tensor_mul(vals[:, 0:1], gm[:, 0:1], vals[:, 1:2])
                bc = gps_pool.tile([128, 2], f32, name=f"bc_{b}_{cch}",
                                   tag="bc")
                nc.tensor.matmul(bc, lhsT=indtg_sb[cch], rhs=vals, start=True,
                                 stop=True)
                ab = tiny.tile([128, 2], f32, name=f"ab_{b}_{cch}", tag="ab")
                nc.vector.tensor_copy(out=ab, in_=bc)
                bt = tiny.tile([128, 1], f32, name=f"bt_{b}_{cch}", tag="bt")
                nc.vector.tensor_sub(bt, gb_sb[cch][:, 1:2], ab[:, 0:1])
                abs_[b][cch] = (ab, bt)

        def emit_act(b, pieces=None, chunks=(0, 1)):
            # Pad tiles carry whole-tile dependencies (strided 3D writes vs
            # flat window reads), so the NEXT image's conv waits for the
            # LAST act piece: uneven pieces put a small piece last to
            # shorten that serial tail, while few pieces keep the per-pass
            # fixed overhead low.  Pieces are emitted piece-outer /
            # chunk-inner so neither chunk head-blocks the other.
            if pieces is None:
                pieces = CFG["pieces"]
            s = b % 2
            tiles = {}
            for cch in chunks:
                tiles[cch] = (
                    t_pool.tile([128, HW], f32, name=f"t_{b}_{cch}",
                                tag=f"t{cch}"),
                    u_pool.tile([128, HW], f16, name=f"u_{b}_{cch}",
                                tag=f"u{cch}"),
                    n_pool.tile([128, HW], f16, name=f"n_{b}_{cch}",
                                tag=f"n{cch}"),
                    g_pool.tile([128, HW], f16, name=f"g_{b}_{cch}",
                                tag=f"g{cch}"),
                    pad3d(hi_t[s], cch), pad3d(lo_t[s], cch))
            starts = [sum(pieces[:i]) for i in range(len(pieces))]
            if CFG["interleave"]:
                order = [(h, cch) for h in range(len(pieces))
                         for cch in chunks]
            else:
                order = [(h, cch) for cch in chunks
                         for h in range(len(pieces))]
            for h, cch in order:
                hrows = pieces[h]
                r0 = starts[h]
                sl = slice(r0 * W, (r0 + hrows) * W)
                rs = slice(1 + r0, 1 + r0 + hrows)
                if True:
                    ab, bt = abs_[b][cch]
                    t, u16, n16, g16, hi3, lo3 = tiles[cch]
                    # t = relu(A*x + B)
                    nc.scalar.activation(out=t[:, sl], in_=xs[b][cch][:, sl],
                                         func=AF.Relu,
                                         bias=bt, scale=ab[:, 1:2])
                    # u = (sqrt(S)*t)^2 = S*relu(z)^2, fp16
                    nc.scalar.activation(out=u16[:, sl], in_=t[:, sl],
                                         func=AF.Square, scale=sqrt_s)
                    # n16 = min(u,255) + 1024: fp32->fp16 RNE convert
                    # rounds to the integer grid (1024 even keeps jnp.round
                    # tie parity); fp16 in/out -> DVE 2x mode
                    nc.vector.tensor_scalar(
                        out=n16[:, sl], in0=u16[:, sl],
                        scalar1=255.0, scalar2=1024.0,
                        op0=ALU.min, op1=ALU.add)
                    # g16 = min(u,255) + 16376 = 16384 + (n' - 8) where
                    # n' = min(u,255): fp16 RNE at the ulp-16 binade rounds
                    # to 16384 + 16*hi, hi = round((n'-8)/16) in [0,15].
                    # Reading u16 directly (not n16) makes g16 and n16
                    # parallel-ready, shortening the act chain tail.  (The
                    # double-rounding n' vs n differs only when u rounds
                    # across the half-integer 16-boundary; r8 absorbs the
                    # difference exactly since r = n - 16*hi in [0,15] +-1
                    # stays fp8-exact in [-1,16].)
                    g_eng = (nc.vector if CFG["g16"] == "dve" else
                             nc.gpsimd if CFG["g16"] == "pool" else
                             (nc.vector if (2 * h + cch) % 2 == 0
                              else nc.gpsimd))
                    g_eng.tensor_scalar(
                        out=g16[:, sl], in0=u16[:, sl],
                        scalar1=255.0, scalar2=16376.0,
                        op0=ALU.min, op1=ALU.add)
                    # hi8 = g16 - 16384: multiples of 16 in [0,240], exact
                    # in fp8e4m3
                    hi_eng = (nc.gpsimd if CFG["hi8"] == "pool" or
                              (2 * h + cch) % 2 == 0 else nc.vector)
                    hi_eng.tensor_scalar_add(
                        out=hi3[:, rs, 1:W + 1],
                        in0=g16[:, sl].rearrange("p (h w) -> p h w",
                                                 h=hrows),
                        scalar1=-16384.0)
                    # r8 = (n16 + 15360) - g16 = n - 16*hi: integers 0..15,
                    # exact in fp8e4m3
                    r8_eng = (nc.vector if CFG["r8"] == "dve" or
                              (2 * h + cch) % 2 == 0 else nc.gpsimd)
                    r8_eng.scalar_tensor_tensor(
                        out=lo3[:, rs, 1:W + 1],
                        in0=n16[:, sl].rearrange("p (h w) -> p h w", h=hrows),
                        scalar=15360.0,
                        in1=g16[:, sl].rearrange("p (h w) -> p h w", h=hrows),
                        op0=ALU.add, op1=ALU.subtract)

        def emit_conv(b, q, evac="pool"):
            # one full-image output tile per (b, q): evacs write slices and
            # a few big DMAs store it
            s = b % 2
            osb = out_pool.tile([128, HW], f16, name=f"o_{b}_{q}", tag="osb",
                                bufs=min(3, CFG["obufs"]))
            for rb in range(RB):
                ps = cps_pool.tile([128, NW], f32, name=f"ps_{b}_{q}_{rb}",
                                   tag="cps")
                i = 0
                parts = ((lo_t[s], hi_t[s]) if CFG["lo_first"]
                         else (hi_t[s], lo_t[s]))
                for pt in parts:
                    for k in range(9):
                        dy, dx = divmod(k, 3)
                        o = (rb * RBH + dy) * PS + dx
                        nc.tensor.matmul(
                            ps,
                            lhsT=w_sb[:, :, k, q, :],
                            rhs=pt[:, :, o:o + NW],
                            start=(i == 0), stop=(i == 17),
                            perf_mode=DR)
                        i += 1
                # evacuate only the 56 valid columns of each 58-wide row.
                # GPSIMD cannot read PSUM (BIR verifier) so evacs go to ACT
                # or DVE; the last image alternates the two per row block to
                # halve the serial tail.
                o3 = osb[:, rb * NN:(rb + 1) * NN].rearrange(
                    "p (r c) -> p r c", c=W)
                p3 = ps.rearrange("p (r c) -> p r c", c=PS)[:, :, 0:W]
                rb_evac = (("act" if rb % 2 == 0 else "dve")
                           if evac == "alt" else
                           ("dve" if rb % 2 == 0 else "act")
                           if evac == "dalt" else
                           ("act" if rb % 3 < 2 else "dve")
                           if evac == "aad" else
                           ("act" if rb % 3 != 1 else "dve")
                           if evac == "ada" else
                           ("dve" if rb % 3 == 0 else "act")
                           if evac == "daa" else evac)
                if rb_evac == "act":
                    nc.scalar.activation(
                        out=o3, in_=p3, func=AF.Identity,
                        bias=sc_sb[q][:, 1:2], scale=sc_sb[q][:, 0:1])
                else:
                    nc.vector.tensor_scalar(
                        out=o3, in0=p3,
                        scalar1=sc_sb[q][:, 0:1], scalar2=sc_sb[q][:, 1:2],
                        op0=ALU.mult, op1=ALU.add)
            # piecewise stores: each piece only depends on the evacs that
            # wrote it (subtile deps).  The last image streams out
            # per-rowblock so the kernel tail isn't gated on one big DMA.
            cuts = (list(range(0, HW + 1, NN)) if b == B_LOC - 1
                    else [0, 4 * NN, HW])
            for lo, hi in zip(cuts[:-1], cuts[1:]):
                nc.sync.dma_start(out=out_ap[b, q * 128:(q + 1) * 128, lo:hi],
                                  in_=osb[:, lo:hi])

        if CFG["prologue"] == "upfront":
            emit_stats_pre(0)
            emit_stats_mm(0)
            emit_post(0)
            emit_act(0, pieces=CFG["pieces0"])
        else:
            for cch in range(2):
                emit_stats_pre(0, chunks=(cch,))
                emit_stats_mm(0, chunks=(cch,))
                emit_post(0, chunks=(cch,))
                emit_act(0, pieces=CFG["pieces0"], chunks=(cch,))
        for b in range(B_LOC):
            last = b + 1 == B_LOC
            if not last:
                emit_load(b + 1,
                          gate=(b == 0 and CFG["gate_load1"]))
                emit_stats_pre(b + 1)
            emit_conv(b, 0, evac=CFG["evac_last"] if last else CFG["evac0"])
            if not last:
                # the next image's whole activation pipeline is emitted
                # between the two half-image convs: its pads (other buffer)
                # are produced while the PE chews on conv(b, 1), so the
                # image transition has no serial act chain on the critical
                # path (subtile deps let conv(b+1) row blocks start as rows
                # land).  conv(b,0) evacs go to GPSIMD (whose queue is
                # otherwise free) and conv(b,1) evacs to ACT (which drains
                # the act passes first, still well within the PSUM cushion)
                # so neither queue blocks the other image's work.
                emit_stats_mm(b + 1)
                emit_post(b + 1)
                emit_act(b + 1)
            emit_conv(b, 1, evac=CFG["evac_last"] if last else CFG["evac1"])

    nc.compile()
    return nc


def kernel(x, gamma, beta, a, weight_fp, bias):
    consts = _host_prep(np.asarray(gamma), np.asarray(beta), np.asarray(a),
                        np.asarray(weight_fp), np.asarray(bias))
    nc = _build_nc(consts.pop("sqrt_s"))

    from concourse.bass_utils import run_bass_kernel_spmd

    x = np.ascontiguousarray(np.asarray(x, dtype=np.float32)
                             .reshape(B_TOT, C, HW).astype(np.float16))
    in_maps = []
    for core in range(N_CORES):
        in_maps.append({
            "x": x[core * B_LOC:(core + 1) * B_LOC],
            "wt": consts["wt"],
            "pack": consts["pack"],
        })
    res = run_bass_kernel_spmd(nc, in_maps, list(range(N_CORES)))
    out = np.concatenate([np.asarray(res.results[i]["out"], dtype=np.float32)
                          for i in range(N_CORES)], axis=0)
    return out.reshape(B_TOT, C, H, W)


if __name__ == "__main__":
    rng = np.random.default_rng(0)
    x = rng.standard_normal((B_TOT, C, H, W), dtype=np.float32)
    out = kernel(x, np.ones(C, np.float32), np.zeros(C, np.float32),
                 np.float32(6.0),
                 rng.standard_normal((C, C, 3, 3), dtype=np.float32) * 0.03,
                 np.zeros(C, np.float32))
    print(out.shape, out.dtype)

